# revision 16
# baseline (speedup 1.0000x reference)
"""Trainium2 Bass kernel for a GQA attention block (B=2, S=2048, H=2048,
16 q-heads / 8 kv-heads, head_dim=128, fp32), tensor-parallel over heads
across 8 NeuronCores.

Per-core shard (core c): q-heads {2c, 2c+1}, kv-head c; wq/wk/wv column
shards, wo row shard. x replicated (pre-transposed on host). Each core
emits a partial [4096, 2048] o-proj product; host sums the 8 partials.

v2 design (all matmul I/O in bf16; graded by the TimelineSim cost model):
  - Emission order A(b0) B(b0) A(b1) B(b1): batch-b attention starts as
    soon as batch-b projections finish; b1's input DMA prefetches under
    B(b0) (loads on the sync queue, output stores on the scalar queue).
  - Phase A per 512-token tile: Q^T/K^T projection slabs [d,tok] plus V
    produced directly in natural [tok,d] layout (x^T chunks stationary),
    so no PE transposes. Rope rotate-half runs on DVE stream_shuffle:
    the head dim is host-permuted so (d, d+64) pairs sit in the same
    32-partition block (within-block shuffle is all HW supports); the
    rotation sign is folded into the sin tables. RMSNorm rstd applied
    post-rope (column-uniform, commutes); the whole mul chain is bf16
    for DVE 2x throughput.
  - Phase B per (b, q-tile): scores S^T [k,q] per 128-k tile; exp on ACT
    (4 k-tiles per instruction); causal masking via gpsimd affine_select
    on the two diagonal-band tiles; softmax denominator as ap=1 matmuls
    (exp tile stationary, ones moving -> [q,1] PSUM columns, ~free on
    the PE); PV accumulated [d,q]; o-proj per head into separate PSUM
    halves, normalization fused into eviction as per-partition scales
    (ACT/DVE scale + DVE/Pool scalar_tensor_tensor accumulate).
"""

import math
import os
import sys

import numpy as np

for _p in ("/opt/trn_rl_repo", "/root/.axon_site/_ro/trn_rl_repo"):
    if os.path.isdir(_p) and _p not in sys.path:
        sys.path.insert(0, _p)
        break

import concourse.bacc as bacc
import concourse.tile as tile
from concourse import mybir
from concourse.bass_isa import ReduceOp
from concourse.bass_utils import run_bass_kernel_spmd
from concourse.masks import make_identity

# Problem constants (hardcoded per contract)
B, S, HID = 2, 2048, 2048
NH, NKV, D = 16, 8, 128
NCORES = 8
HQ = NH // NCORES  # q heads per core = 2
T = B * S          # 4096 tokens
EPS = 1e-5
F32 = mybir.dt.float32
BF16 = mybir.dt.bfloat16
SCALE = 1.0 / math.sqrt(D)

KT = HID // 128      # 16 contraction tiles
QT_PER_B = S // 512  # 4 q-tiles per batch

# within-32-block half-rotation (self-inverse); with the host-side head-dim
# permutation below this realizes rotate-half across the full 128 dims
SHUF_MASK = [(i + 16) % 32 for i in range(32)]


def head_perm():
    """p -> d: partition p holds original head-dim d; pairs (d, d+64) share
    a 32-partition block 16 apart, so stream_shuffle can swap them."""
    perm = np.zeros(D, dtype=np.int64)
    for p in range(D):
        blk, i = p // 32, p % 32
        perm[p] = 16 * blk + (i % 16) + 64 * (i // 16)
    return perm


def build_nc():
    nc = bacc.Bacc("TRN2", target_bir_lowering=False, debug=False)
    xt = nc.dram_tensor("xt", [HID, T], BF16, kind="ExternalInput").ap()
    wqkv = nc.dram_tensor("wqkv", [HID, 4 * D], BF16, kind="ExternalInput").ap()
    woc = nc.dram_tensor("woc", [HQ * D, HID], BF16, kind="ExternalInput").ap()
    onec = nc.dram_tensor("onec", [D, 1], BF16, kind="ExternalInput").ap()
    ctq = nc.dram_tensor("ctq", [D, S], BF16, kind="ExternalInput").ap()
    stq = nc.dram_tensor("stq", [D, S], BF16, kind="ExternalInput").ap()
    ctk = nc.dram_tensor("ctk", [D, S], BF16, kind="ExternalInput").ap()
    stk = nc.dram_tensor("stk", [D, S], BF16, kind="ExternalInput").ap()
    out = nc.dram_tensor("out", [T, HID], F32, kind="ExternalOutput").ap()

    with tile.TileContext(nc) as tc:
        from contextlib import ExitStack

        with ExitStack() as root:
            const = root.enter_context(tc.tile_pool(name="const", bufs=1))
            ident = const.tile([128, 128], F32, name="ident")
            make_identity(nc, ident)
            ones_col = const.tile([128, 1], BF16, name="ones_col")
            nc.scalar.dma_start(out=ones_col, in_=onec)
            eps_col = const.tile([128, 1], F32, name="eps_col")
            nc.vector.memset(eps_col, EPS)

            res = root.enter_context(tc.tile_pool(name="res", bufs=1))
            wo_sb = res.tile([128, HQ, HID], BF16, name="wo_sb")
            wqkv_sb = res.tile([128, KT, 4 * D], BF16, name="wqkv_sb")
            qt_sb = res.tile([128, HQ, T], BF16, name="qt_sb")   # [d, h, tok]
            kt_sb = res.tile([128, T], BF16, name="kt_sb")       # [d, tok]
            v_sb = res.tile([128, T // 128, D], BF16, name="v_sb")
            tabs = {}
            for nm in ("cq", "sq", "ck", "sk"):
                tabs[nm] = res.tile([128, S], BF16, name="tab_" + nm)

            xp = root.enter_context(tc.tile_pool(name="xp", bufs=18))
            wp = root.enter_context(tc.tile_pool(name="wp", bufs=2))
            ep = root.enter_context(tc.tile_pool(name="ep", bufs=19))
            atp = root.enter_context(tc.tile_pool(name="atp", bufs=8))
            rdp = root.enter_context(tc.tile_pool(name="rdp", bufs=8))
            rrp = root.enter_context(tc.tile_pool(name="rrp", bufs=4))
            rbp = root.enter_context(tc.tile_pool(name="rbp", bufs=4))
            op = root.enter_context(tc.tile_pool(name="op", bufs=3))
            psB = root.enter_context(tc.tile_pool(name="psB", bufs=2, space="PSUM"))
            psPo = root.enter_context(tc.tile_pool(name="psPo", bufs=2, space="PSUM"))
            psO = root.enter_context(tc.tile_pool(name="psO", bufs=2, space="PSUM"))

            def phase_a(b):
                """Projections + norm + rope for batch b's 4 token tiles."""
                xhs = {}

                def load_x(k, half):
                    xh = xp.tile([128, 1024], BF16, name="xh", tag="xh")
                    nc.sync.dma_start(
                        out=xh,
                        in_=xt[k * 128:(k + 1) * 128,
                               b * S + half * 1024: b * S + (half + 1) * 1024],
                    )
                    xhs[(k, half)] = xh

                # first-half x loads, with weight loads interleaved (b==0) so
                # the first slab's deps resolve early
                for k0 in range(0, KT, 4):
                    if b == 0:
                        nc.sync.dma_start(
                            out=wqkv_sb[:, k0:k0 + 4, :],
                            in_=wqkv[k0 * 128:(k0 + 4) * 128, :].rearrange(
                                "(k p) n -> p k n", p=128),
                        )
                    for k in range(k0, k0 + 4):
                        load_x(k, 0)
                if b == 0:
                    for nm, ap in (("cq", ctq), ("sq", stq), ("ck", ctk),
                                   ("sk", stk)):
                        nc.sync.dma_start(out=tabs[nm], in_=ap)
                    nc.sync.dma_start(
                        out=wo_sb, in_=woc.rearrange("(h p) n -> p h n", p=128))
                for k in range(KT):
                    load_x(k, 1)
                for tl in range(4):  # local 512-token tile
                    t = b * 4 + tl
                    xks = [xhs[(k, tl // 2)][:, (tl % 2) * 512:(tl % 2 + 1) * 512]
                           for k in range(KT)]
                    # two PSUM slabs: (q0,q1) and (k, v-natural)
                    big_q = psB.tile([128, 1024], F32, name="ps_q", tag="ps")
                    for m in range(2):
                        dst = big_q[:, m * 512:(m + 1) * 512]
                        for k in range(KT):
                            nc.tensor.matmul(
                                dst, lhsT=wqkv_sb[:, k, m * 128:(m + 1) * 128],
                                rhs=xks[k], start=(k == 0), stop=(k == KT - 1),
                            )
                    big_kv = psB.tile([128, 1024], F32, name="ps_kv", tag="ps")
                    for k in range(KT):
                        nc.tensor.matmul(
                            big_kv[:, 0:512], lhsT=wqkv_sb[:, k, 256:384],
                            rhs=xks[k], start=(k == 0), stop=(k == KT - 1),
                        )
                    for k in range(KT):
                        for j in range(4):
                            nc.tensor.matmul(
                                big_kv[:, 512 + j * 128:512 + (j + 1) * 128],
                                lhsT=xks[k][:, j * 128:(j + 1) * 128],
                                rhs=wqkv_sb[:, k, 384:512],
                                start=(k == 0), stop=(k == KT - 1),
                                skip_group_check=True,
                            )
                    nc.scalar.copy(v_sb[:, t * 4:(t + 1) * 4, :],
                                   big_kv[:, 512:1024])

                    s0 = tl * 512  # position-in-sequence
                    for m, src, cosT, sinT in (
                        (0, big_q[:, 0:512], tabs["cq"], tabs["sq"]),
                        (1, big_q[:, 512:1024], tabs["cq"], tabs["sq"]),
                        (2, big_kv[:, 0:512], tabs["ck"], tabs["sk"]),
                    ):
                        cosT = cosT[:, s0:s0 + 512]
                        sinT = sinT[:, s0:s0 + 512]
                        qk = wp.tile([128, 512], BF16, name="qk", tag="qk")
                        nc.scalar.copy(qk, src)  # sole PSUM reader (ACT)
                        sq = wp.tile([128, 512], BF16, name="sqr", tag="sqr")
                        nc.vector.tensor_mul(sq, qk, qk)
                        nc.gpsimd.partition_all_reduce(sq, sq, 128, ReduceOp.add)
                        rrow = wp.tile([1, 512], F32, name="rrow", tag="rrow")
                        nc.scalar.activation(
                            rrow, sq[0:1, :], mybir.ActivationFunctionType.Sqrt,
                            bias=eps_col[0:1, :], scale=1.0 / D,
                        )
                        rrow_r = wp.tile([1, 512], BF16, name="rrow_r", tag="rrow_r")
                        with nc.allow_low_precision(
                                reason="rstd in bf16; rel-err budget 2e-2"):
                            nc.vector.reciprocal(rrow_r, rrow)
                        rstd = wp.tile([128, 512], BF16, name="rstd", tag="rstd")
                        nc.gpsimd.partition_broadcast(rstd, rrow_r)
                        shf = wp.tile([128, 512], BF16, name="shf", tag="shf")
                        nc.vector.stream_shuffle(shf, qk, SHUF_MASK)
                        t0 = wp.tile([128, 512], BF16, name="t0", tag="t0")
                        nc.vector.tensor_mul(t0, qk, cosT)
                        t1 = wp.tile([128, 512], BF16, name="t1", tag="t1")
                        nc.vector.tensor_mul(t1, shf, sinT)
                        tr = wp.tile([128, 512], BF16, name="tr", tag="tr")
                        nc.vector.tensor_add(tr, t0, t1)
                        if m < 2:
                            dst = qt_sb[:, m, t * 512:(t + 1) * 512]
                        else:
                            dst = kt_sb[:, t * 512:(t + 1) * 512]
                        nc.vector.tensor_mul(dst, tr, rstd)

            def phase_b(b):
                """Causal attention + row-parallel o-proj partial, batch b.

                Software-pipelined: group i+1's scores are emitted before
                group i's PV so the in-order PE never idles on the ACT exp
                latency; each q-tile's o-proj is deferred behind the next
                group's scores for the same reason.
                """
                at_tiles = {}
                rd_tiles = {}

                def scores(qt, h, qh):
                    qq0 = qt * 512 + qh * 256
                    n_kt = (qq0 + 256) // 128
                    ets = []
                    for k0 in range(0, n_kt, 4):
                        kn = min(4, n_kt - k0)
                        st = psB.tile([128, 1024], F32, name="st", tag="ps")
                        for j in range(kn):
                            kt = k0 + j
                            nc.tensor.matmul(
                                st[:, j * 256:(j + 1) * 256],
                                lhsT=kt_sb[:, b * S + kt * 128:
                                           b * S + (kt + 1) * 128],
                                rhs=qt_sb[:, h, b * S + qq0: b * S + qq0 + 256],
                                start=True, stop=True,
                                skip_group_check=True,
                            )
                        et = ep.tile([128, 1024], BF16, name="et", tag="et")
                        nc.scalar.activation(
                            et[:, 0:kn * 256], st[:, 0:kn * 256],
                            mybir.ActivationFunctionType.Exp, scale=SCALE,
                        )
                        for j in range(kn):
                            kt = k0 + j
                            esl = et[:, j * 256:(j + 1) * 256]
                            if kt * 128 + 127 > qq0:  # diagonal band
                                nc.gpsimd.affine_select(
                                    out=esl, in_=esl,
                                    pattern=[[1, 256]],
                                    channel_multiplier=-1,
                                    base=-(kt * 128 - qq0),
                                    compare_op=mybir.AluOpType.is_ge,
                                    fill=0.0,
                                )
                            ets.append(esl)
                    return ets

                def pv(qt, h, qh, ets):
                    """PV + den matmuls + reciprocal; returns a finisher that
                    normalizes the PV output into at_tiles. The finisher is
                    deferred one group so the PE transpose and the DVE/Pool
                    chain never head-of-line block the PE."""
                    n_kt = len(ets)
                    od = psO.tile([128, 512], F32, name="od", tag="od")
                    for kt in range(n_kt):
                        nc.tensor.matmul(
                            od[:, 0:256],
                            lhsT=v_sb[:, b * (S // 128) + kt, :],
                            rhs=ets[kt],
                            start=(kt == 0), stop=(kt == n_kt - 1),
                            skip_group_check=True,
                        )
                        for c in range(2):
                            nc.tensor.matmul(
                                od[:, 256 + c:257 + c],
                                lhsT=ets[kt][:, c * 128:(c + 1) * 128],
                                rhs=ones_col,
                                start=(kt == 0), stop=(kt == n_kt - 1),
                                skip_group_check=True,
                            )
                    r_t = rdp.tile([128, 2], F32, name="rd", tag="rd")
                    nc.vector.reciprocal(r_t, od[:, 256:258])

                    def fin():
                        # den cols are consumed; overwrite with rows rr[1,256]
                        for c in range(2):
                            nc.tensor.transpose(
                                od[0:1, 256 + c * 128:384 + c * 128],
                                r_t[:, c:c + 1], ident)
                        rr = rrp.tile([1, 256], F32, name="rr", tag="rr")
                        nc.vector.tensor_copy(rr, od[0:1, 256:512])
                        rb = rbp.tile([128, 256], F32, name="rb", tag="rb")
                        for c in range(2):
                            nc.gpsimd.partition_broadcast(
                                rb[:, c * 128:(c + 1) * 128],
                                rr[:, c * 128:(c + 1) * 128])
                        a_t = atp.tile([128, 256], BF16, name="at", tag="at")
                        nc.vector.tensor_mul(a_t, od[:, 0:256], rb)
                        at_tiles[(h, qh)] = a_t

                    return fin

                def oproj(qt, ats):
                    for mq in range(4):
                        qh, c = mq // 2, mq % 2
                        ob = op.tile([128, 2048], F32, name="ob", tag="ob")
                        for nn in range(4):
                            po = psPo.tile([128, 512], F32, name="po", tag="po")
                            for h in range(HQ):
                                nc.tensor.matmul(
                                    po,
                                    lhsT=ats[(h, qh)][:, c * 128:(c + 1) * 128],
                                    rhs=wo_sb[:, h, nn * 512:(nn + 1) * 512],
                                    start=(h == 0), stop=(h == HQ - 1),
                                )
                            osl = ob[:, nn * 512:(nn + 1) * 512]
                            # alternate Pool/DVE so consecutive po evictions
                            # overlap and ACT keeps feeding exps to the PE
                            if nn % 2 == 0:
                                nc.gpsimd.tensor_copy(osl, po)
                            else:
                                nc.vector.tensor_copy(osl, po)
                        nc.scalar.dma_start(
                            out=out[b * S + qt * 512 + mq * 128:
                                    b * S + qt * 512 + (mq + 1) * 128, :],
                            in_=ob,
                        )

                # super-iteration pipeline: oproj(qt-2) | per-group scores(qt)
                # + finisher(prev group) + pv(qt-1). Every engine's in-order
                # stream stays in true execution order: late-dep work (oproj
                # evictions, finisher chains) always sits behind the PE work
                # it depends on, so no head-of-line blocking.
                groups = [(h, qh) for h in range(HQ) for qh in range(2)]
                pend_ets = {}
                pend_fin = None
                for qt in range(QT_PER_B + 2):
                    if qt >= 2:
                        if pend_fin is not None:  # last fin of qt-1's pvs —
                            pend_fin()            # completes qt-2's at set
                            pend_fin = None
                        oproj(qt - 2, at_tiles)   # at_tiles == qt-2's tiles
                    for g in groups:
                        if qt < QT_PER_B:
                            pend_ets[(qt, *g)] = scores(qt, *g)
                        new_fin = None
                        if 1 <= qt <= QT_PER_B:
                            new_fin = pv(qt - 1, *g, pend_ets.pop((qt - 1, *g)))
                        # fin(g-1) emits after pv(g)'s matmuls: its PE
                        # transposes then never wait on the DVE reciprocal
                        if pend_fin is not None:
                            pend_fin()
                        pend_fin = new_fin

            for b in range(B):
                phase_a(b)
                phase_b(b)

    nc.compile()
    return nc


def prep_inputs(x, cos, sin, wq, wk, wv, wo, q_norm_w, k_norm_w):
    """Host-side sharding/layout prep. Returns per-core in_maps."""
    import ml_dtypes
    f = np.float32
    bf = np.dtype(ml_dtypes.bfloat16)
    cvt = lambda a: np.ascontiguousarray(np.asarray(a, f).astype(bf))
    x = np.asarray(x, f)
    cos = np.asarray(cos, f)
    sin = np.asarray(sin, f)
    wq, wk, wv, wo = (np.asarray(a, f) for a in (wq, wk, wv, wo))
    q_norm_w = np.asarray(q_norm_w, f)
    k_norm_w = np.asarray(k_norm_w, f)

    perm = head_perm()                      # partition p holds dim perm[p]
    partner = np.array([(p // 32) * 32 + ((p % 32) + 16) % 32
                        for p in range(D)])  # stream_shuffle pairing
    sign = np.where(perm[np.arange(D)] < D // 2, -1.0, 1.0).astype(f)
    # rot_half weight fold: t1[p] = qk[partner(p)] * stq[p];
    # stq[p] = sign(d_p) * sin[d_p] * w[d at partner]
    xt = np.ascontiguousarray(x.reshape(T, HID).T)  # [HID, T]
    ctq = cos.T[perm] * q_norm_w[perm][:, None]
    stq = sin.T[perm] * q_norm_w[perm[partner]][:, None] * sign[:, None]
    ctk = cos.T[perm] * k_norm_w[perm][:, None]
    stk = sin.T[perm] * k_norm_w[perm[partner]][:, None] * sign[:, None]
    onec = np.ones((D, 1), f)
    xt_m, ctq_m, stq_m, ctk_m, stk_m, onec_m = (
        cvt(a) for a in (xt, ctq, stq, ctk, stk, onec))

    in_maps = []
    for c in range(NCORES):
        wq_c = wq[:, c * HQ * D:(c + 1) * HQ * D].reshape(HID, HQ, D)
        wq_c = wq_c[:, :, perm].reshape(HID, HQ * D)  # permuted head dims
        wk_c = wk[:, c * D:(c + 1) * D][:, perm]
        wv_c = wv[:, c * D:(c + 1) * D]               # v unpermuted
        wqkv_c = np.ascontiguousarray(
            np.concatenate([wq_c, wk_c, wv_c], axis=1))
        woc = np.ascontiguousarray(wo[c * HQ * D:(c + 1) * HQ * D, :])
        in_maps.append({
            "xt": xt_m, "wqkv": cvt(wqkv_c), "woc": cvt(woc), "onec": onec_m,
            "ctq": ctq_m, "stq": stq_m, "ctk": ctk_m, "stk": stk_m,
        })
    return in_maps


_NC = None


def get_nc():
    global _NC
    if _NC is None:
        _NC = build_nc()
    return _NC


def kernel(x, cos, sin, wq, wk, wv, wo, q_norm_w, k_norm_w):
    nc = get_nc()
    in_maps = prep_inputs(x, cos, sin, wq, wk, wv, wo, q_norm_w, k_norm_w)
    res = run_bass_kernel_spmd(nc, in_maps, core_ids=list(range(NCORES)))
    acc = np.zeros((T, HID), dtype=np.float64)
    for c in range(NCORES):
        acc += res.results[c]["out"]
    return acc.astype(np.float32).reshape(B, S, HID)


# revision 17
# speedup vs baseline: 1.0186x; 1.0186x over previous
"""Trainium2 Bass kernel for a GQA attention block (B=2, S=2048, H=2048,
16 q-heads / 8 kv-heads, head_dim=128, fp32), tensor-parallel over heads
across 8 NeuronCores.

Per-core shard (core c): q-heads {2c, 2c+1}, kv-head c; wq/wk/wv column
shards, wo row shard. x replicated (pre-transposed on host). Each core
emits a partial [4096, 2048] o-proj product; host sums the 8 partials.

v2 design (all matmul I/O in bf16; graded by the TimelineSim cost model):
  - Emission order A(b0) B(b0) A(b1) B(b1): batch-b attention starts as
    soon as batch-b projections finish; b1's input DMA prefetches under
    B(b0) (loads on the sync queue, output stores on the scalar queue).
  - Phase A per 512-token tile: Q^T/K^T projection slabs [d,tok] plus V
    produced directly in natural [tok,d] layout (x^T chunks stationary),
    so no PE transposes. Rope rotate-half runs on DVE stream_shuffle:
    the head dim is host-permuted so (d, d+64) pairs sit in the same
    32-partition block (within-block shuffle is all HW supports); the
    rotation sign is folded into the sin tables. RMSNorm rstd applied
    post-rope (column-uniform, commutes); the whole mul chain is bf16
    for DVE 2x throughput.
  - Phase B per (b, q-tile): scores S^T [k,q] per 128-k tile; exp on ACT
    (4 k-tiles per instruction); causal masking via gpsimd affine_select
    on the two diagonal-band tiles; softmax denominator as ap=1 matmuls
    (exp tile stationary, ones moving -> [q,1] PSUM columns, ~free on
    the PE); PV accumulated [d,q]; o-proj per head into separate PSUM
    halves, normalization fused into eviction as per-partition scales
    (ACT/DVE scale + DVE/Pool scalar_tensor_tensor accumulate).
"""

import math
import os
import sys

import numpy as np

for _p in ("/opt/trn_rl_repo", "/root/.axon_site/_ro/trn_rl_repo"):
    if os.path.isdir(_p) and _p not in sys.path:
        sys.path.insert(0, _p)
        break

import concourse.bacc as bacc
import concourse.tile as tile
from concourse import mybir
from concourse.bass_isa import ReduceOp
from concourse.bass_utils import run_bass_kernel_spmd
from concourse.masks import make_identity

# Problem constants (hardcoded per contract)
B, S, HID = 2, 2048, 2048
NH, NKV, D = 16, 8, 128
NCORES = 8
HQ = NH // NCORES  # q heads per core = 2
T = B * S          # 4096 tokens
EPS = 1e-5
F32 = mybir.dt.float32
BF16 = mybir.dt.bfloat16
SCALE = 1.0 / math.sqrt(D)

KT = HID // 128      # 16 contraction tiles
QT_PER_B = S // 512  # 4 q-tiles per batch

# within-32-block half-rotation (self-inverse); with the host-side head-dim
# permutation below this realizes rotate-half across the full 128 dims
SHUF_MASK = [(i + 16) % 32 for i in range(32)]


def head_perm():
    """p -> d: partition p holds original head-dim d; pairs (d, d+64) share
    a 32-partition block 16 apart, so stream_shuffle can swap them."""
    perm = np.zeros(D, dtype=np.int64)
    for p in range(D):
        blk, i = p // 32, p % 32
        perm[p] = 16 * blk + (i % 16) + 64 * (i // 16)
    return perm


def build_nc():
    nc = bacc.Bacc("TRN2", target_bir_lowering=False, debug=False)
    xt = nc.dram_tensor("xt", [HID, T], BF16, kind="ExternalInput").ap()
    wqkv = nc.dram_tensor("wqkv", [HID, 4 * D], BF16, kind="ExternalInput").ap()
    woc = nc.dram_tensor("woc", [HQ * D, HID], BF16, kind="ExternalInput").ap()
    onec = nc.dram_tensor("onec", [D, 1], BF16, kind="ExternalInput").ap()
    ctq = nc.dram_tensor("ctq", [D, S], BF16, kind="ExternalInput").ap()
    stq = nc.dram_tensor("stq", [D, S], BF16, kind="ExternalInput").ap()
    ctk = nc.dram_tensor("ctk", [D, S], BF16, kind="ExternalInput").ap()
    stk = nc.dram_tensor("stk", [D, S], BF16, kind="ExternalInput").ap()
    out = nc.dram_tensor("out", [T, HID], F32, kind="ExternalOutput").ap()

    with tile.TileContext(nc) as tc:
        from contextlib import ExitStack

        with ExitStack() as root:
            const = root.enter_context(tc.tile_pool(name="const", bufs=1))
            ident = const.tile([128, 128], F32, name="ident")
            make_identity(nc, ident)
            ones_col = const.tile([128, 1], BF16, name="ones_col")
            nc.scalar.dma_start(out=ones_col, in_=onec)
            eps_col = const.tile([128, 1], F32, name="eps_col")
            nc.vector.memset(eps_col, EPS)

            res = root.enter_context(tc.tile_pool(name="res", bufs=1))
            wo_sb = res.tile([128, HQ, HID], BF16, name="wo_sb")
            wqkv_sb = res.tile([128, KT, 4 * D], BF16, name="wqkv_sb")
            qt_sb = res.tile([128, HQ, T], BF16, name="qt_sb")   # [d, h, tok]
            kt_sb = res.tile([128, T], BF16, name="kt_sb")       # [d, tok]
            v_sb = res.tile([128, T // 128, D], BF16, name="v_sb")
            tabs = {}
            for nm in ("cq", "sq", "ck", "sk"):
                tabs[nm] = res.tile([128, S], BF16, name="tab_" + nm)

            xp = root.enter_context(tc.tile_pool(name="xp", bufs=18))
            wp = root.enter_context(tc.tile_pool(name="wp", bufs=2))
            ep = root.enter_context(tc.tile_pool(name="ep", bufs=19))
            atp = root.enter_context(tc.tile_pool(name="atp", bufs=8))
            rdp = root.enter_context(tc.tile_pool(name="rdp", bufs=8))
            rrp = root.enter_context(tc.tile_pool(name="rrp", bufs=4))
            rbp = root.enter_context(tc.tile_pool(name="rbp", bufs=4))
            op = root.enter_context(tc.tile_pool(name="op", bufs=3))
            psB = root.enter_context(tc.tile_pool(name="psB", bufs=2, space="PSUM"))
            psPo = root.enter_context(tc.tile_pool(name="psPo", bufs=2, space="PSUM"))
            psO = root.enter_context(tc.tile_pool(name="psO", bufs=2, space="PSUM"))

            def phase_a(b):
                """Projections + norm + rope for batch b's 4 token tiles."""
                xhs = {}

                def load_x(k, half):
                    xh = xp.tile([128, 1024], BF16, name="xh", tag="xh")
                    nc.sync.dma_start(
                        out=xh,
                        in_=xt[k * 128:(k + 1) * 128,
                               b * S + half * 1024: b * S + (half + 1) * 1024],
                    )
                    xhs[(k, half)] = xh

                # first-half x loads, with weight loads interleaved (b==0) so
                # the first slab's deps resolve early
                for k0 in range(0, KT, 4):
                    if b == 0:
                        nc.sync.dma_start(
                            out=wqkv_sb[:, k0:k0 + 4, :],
                            in_=wqkv[k0 * 128:(k0 + 4) * 128, :].rearrange(
                                "(k p) n -> p k n", p=128),
                        )
                    for k in range(k0, k0 + 4):
                        load_x(k, 0)
                if b == 0:
                    for nm, ap in (("cq", ctq), ("sq", stq), ("ck", ctk),
                                   ("sk", stk)):
                        nc.sync.dma_start(out=tabs[nm], in_=ap)
                    nc.sync.dma_start(
                        out=wo_sb, in_=woc.rearrange("(h p) n -> p h n", p=128))
                for k in range(KT):
                    load_x(k, 1)
                for tl in range(4):  # local 512-token tile
                    t = b * 4 + tl
                    xks = [xhs[(k, tl // 2)][:, (tl % 2) * 512:(tl % 2 + 1) * 512]
                           for k in range(KT)]
                    # two PSUM slabs: (q0,q1) and (k, v-natural)
                    big_q = psB.tile([128, 1024], F32, name="ps_q", tag="ps")
                    for m in range(2):
                        dst = big_q[:, m * 512:(m + 1) * 512]
                        for k in range(KT):
                            nc.tensor.matmul(
                                dst, lhsT=wqkv_sb[:, k, m * 128:(m + 1) * 128],
                                rhs=xks[k], start=(k == 0), stop=(k == KT - 1),
                            )
                    big_kv = psB.tile([128, 1024], F32, name="ps_kv", tag="ps")
                    for k in range(KT):
                        nc.tensor.matmul(
                            big_kv[:, 0:512], lhsT=wqkv_sb[:, k, 256:384],
                            rhs=xks[k], start=(k == 0), stop=(k == KT - 1),
                        )
                    for k in range(KT):
                        for j in range(4):
                            nc.tensor.matmul(
                                big_kv[:, 512 + j * 128:512 + (j + 1) * 128],
                                lhsT=xks[k][:, j * 128:(j + 1) * 128],
                                rhs=wqkv_sb[:, k, 384:512],
                                start=(k == 0), stop=(k == KT - 1),
                                skip_group_check=True,
                            )
                    nc.scalar.copy(v_sb[:, t * 4:(t + 1) * 4, :],
                                   big_kv[:, 512:1024])

                    s0 = tl * 512  # position-in-sequence
                    for m, src, cosT, sinT in (
                        (0, big_q[:, 0:512], tabs["cq"], tabs["sq"]),
                        (1, big_q[:, 512:1024], tabs["cq"], tabs["sq"]),
                        (2, big_kv[:, 0:512], tabs["ck"], tabs["sk"]),
                    ):
                        cosT = cosT[:, s0:s0 + 512]
                        sinT = sinT[:, s0:s0 + 512]
                        qk = wp.tile([128, 512], BF16, name="qk", tag="qk")
                        nc.scalar.copy(qk, src)  # sole PSUM reader (ACT)
                        sq = wp.tile([128, 512], BF16, name="sqr", tag="sqr")
                        nc.vector.tensor_mul(sq, qk, qk)
                        nc.gpsimd.partition_all_reduce(sq, sq, 128, ReduceOp.add)
                        rrow = wp.tile([1, 512], F32, name="rrow", tag="rrow")
                        nc.scalar.activation(
                            rrow, sq[0:1, :], mybir.ActivationFunctionType.Sqrt,
                            bias=eps_col[0:1, :], scale=1.0 / D,
                        )
                        rrow_r = wp.tile([1, 512], BF16, name="rrow_r", tag="rrow_r")
                        with nc.allow_low_precision(
                                reason="rstd in bf16; rel-err budget 2e-2"):
                            nc.vector.reciprocal(rrow_r, rrow)
                        rstd = wp.tile([128, 512], BF16, name="rstd", tag="rstd")
                        nc.gpsimd.partition_broadcast(rstd, rrow_r)
                        shf = wp.tile([128, 512], BF16, name="shf", tag="shf")
                        nc.vector.stream_shuffle(shf, qk, SHUF_MASK)
                        t0 = wp.tile([128, 512], BF16, name="t0", tag="t0")
                        nc.vector.tensor_mul(t0, qk, cosT)
                        t1 = wp.tile([128, 512], BF16, name="t1", tag="t1")
                        nc.vector.tensor_mul(t1, shf, sinT)
                        tr = wp.tile([128, 512], BF16, name="tr", tag="tr")
                        nc.vector.tensor_add(tr, t0, t1)
                        if m < 2:
                            dst = qt_sb[:, m, t * 512:(t + 1) * 512]
                        else:
                            dst = kt_sb[:, t * 512:(t + 1) * 512]
                        nc.vector.tensor_mul(dst, tr, rstd)

            def phase_b(b):
                """Causal attention + row-parallel o-proj partial, batch b.

                Software-pipelined: group i+1's scores are emitted before
                group i's PV so the in-order PE never idles on the ACT exp
                latency; each q-tile's o-proj is deferred behind the next
                group's scores for the same reason.
                """
                at_tiles = {}
                rd_tiles = {}

                def scores(qt, h, qh):
                    qq0 = qt * 512 + qh * 256
                    n_kt = (qq0 + 256) // 128
                    ets = []
                    for k0 in range(0, n_kt, 4):
                        kn = min(4, n_kt - k0)
                        st = psB.tile([128, 1024], F32, name="st", tag="ps")
                        for j in range(kn):
                            kt = k0 + j
                            nc.tensor.matmul(
                                st[:, j * 256:(j + 1) * 256],
                                lhsT=kt_sb[:, b * S + kt * 128:
                                           b * S + (kt + 1) * 128],
                                rhs=qt_sb[:, h, b * S + qq0: b * S + qq0 + 256],
                                start=True, stop=True,
                                skip_group_check=True,
                            )
                        et = ep.tile([128, 1024], BF16, name="et", tag="et")
                        nc.scalar.activation(
                            et[:, 0:kn * 256], st[:, 0:kn * 256],
                            mybir.ActivationFunctionType.Exp, scale=SCALE,
                        )
                        for j in range(kn):
                            kt = k0 + j
                            esl = et[:, j * 256:(j + 1) * 256]
                            if kt * 128 + 127 > qq0:  # diagonal band
                                nc.gpsimd.affine_select(
                                    out=esl, in_=esl,
                                    pattern=[[1, 256]],
                                    channel_multiplier=-1,
                                    base=-(kt * 128 - qq0),
                                    compare_op=mybir.AluOpType.is_ge,
                                    fill=0.0,
                                )
                            ets.append(esl)
                    return ets

                def pv(qt, h, qh, ets):
                    """PV + den matmuls + reciprocal; returns a finisher that
                    normalizes the PV output into at_tiles. The finisher is
                    deferred one group so the PE transpose and the DVE/Pool
                    chain never head-of-line block the PE."""
                    n_kt = len(ets)
                    od = psO.tile([128, 512], F32, name="od", tag="od")
                    for kt in range(n_kt):
                        nc.tensor.matmul(
                            od[:, 0:256],
                            lhsT=v_sb[:, b * (S // 128) + kt, :],
                            rhs=ets[kt],
                            start=(kt == 0), stop=(kt == n_kt - 1),
                            skip_group_check=True,
                        )
                        for c in range(2):
                            nc.tensor.matmul(
                                od[:, 256 + c:257 + c],
                                lhsT=ets[kt][:, c * 128:(c + 1) * 128],
                                rhs=ones_col,
                                start=(kt == 0), stop=(kt == n_kt - 1),
                                skip_group_check=True,
                            )
                    r_t = rdp.tile([128, 2], F32, name="rd", tag="rd")
                    nc.vector.reciprocal(r_t, od[:, 256:258])

                    def fin():
                        # den cols are consumed; overwrite with rows rr[1,256]
                        for c in range(2):
                            nc.tensor.transpose(
                                od[0:1, 256 + c * 128:384 + c * 128],
                                r_t[:, c:c + 1], ident)
                        rr = rrp.tile([1, 256], F32, name="rr", tag="rr")
                        nc.vector.tensor_copy(rr, od[0:1, 256:512])
                        rb = rbp.tile([128, 256], F32, name="rb", tag="rb")
                        for c in range(2):
                            nc.gpsimd.partition_broadcast(
                                rb[:, c * 128:(c + 1) * 128],
                                rr[:, c * 128:(c + 1) * 128])
                        a_t = atp.tile([128, 256], BF16, name="at", tag="at")
                        nc.vector.tensor_mul(a_t, od[:, 0:256], rb)
                        at_tiles[(h, qh)] = a_t

                    return fin

                def oproj(qt, ats):
                    for mq in range(4):
                        qh, c = mq // 2, mq % 2
                        ob = op.tile([128, 2048], F32, name="ob", tag="ob")
                        for nn in range(4):
                            po = psPo.tile([128, 512], F32, name="po", tag="po")
                            for h in range(HQ):
                                nc.tensor.matmul(
                                    po,
                                    lhsT=ats[(h, qh)][:, c * 128:(c + 1) * 128],
                                    rhs=wo_sb[:, h, nn * 512:(nn + 1) * 512],
                                    start=(h == 0), stop=(h == HQ - 1),
                                )
                            osl = ob[:, nn * 512:(nn + 1) * 512]
                            # alternate Pool/DVE so consecutive po evictions
                            # overlap and ACT keeps feeding exps to the PE
                            if nn % 2 == 0:
                                nc.gpsimd.tensor_copy(osl, po)
                            else:
                                nc.vector.tensor_copy(osl, po)
                        nc.scalar.dma_start(
                            out=out[b * S + qt * 512 + mq * 128:
                                    b * S + qt * 512 + (mq + 1) * 128, :],
                            in_=ob,
                        )

                # super-iteration pipeline: oproj(qt-2) | per-group scores(qt)
                # + finisher(prev group) + pv(qt-1). Every engine's in-order
                # stream stays in true execution order: late-dep work (oproj
                # evictions, finisher chains) always sits behind the PE work
                # it depends on, so no head-of-line blocking.
                groups = [(h, qh) for h in range(HQ) for qh in range(2)]
                pend_ets = {}
                pend_fin = None
                for qt in range(QT_PER_B + 2):
                    if qt >= 2 and pend_fin is not None:
                        pend_fin()  # last fin of qt-1's pvs — completes
                        pend_fin = None  # the qt-2 at set early
                    for gi, g in enumerate(groups):
                        if qt < QT_PER_B:
                            pend_ets[(qt, *g)] = scores(qt, *g)
                        new_fin = None
                        if 1 <= qt <= QT_PER_B:
                            new_fin = pv(qt - 1, *g, pend_ets.pop((qt - 1, *g)))
                        # fin(g-1) emits after pv(g)'s matmuls: its PE
                        # transposes then never wait on the DVE reciprocal
                        if pend_fin is not None:
                            pend_fin()
                        pend_fin = new_fin
                        if gi == 0 and qt >= 2:
                            # a full group after the at set completed, so the
                            # po matmuls never wait on the last at-mul
                            oproj(qt - 2, at_tiles)

            for b in range(B):
                phase_a(b)
                phase_b(b)

    nc.compile()
    return nc


def prep_inputs(x, cos, sin, wq, wk, wv, wo, q_norm_w, k_norm_w):
    """Host-side sharding/layout prep. Returns per-core in_maps."""
    import ml_dtypes
    f = np.float32
    bf = np.dtype(ml_dtypes.bfloat16)
    cvt = lambda a: np.ascontiguousarray(np.asarray(a, f).astype(bf))
    x = np.asarray(x, f)
    cos = np.asarray(cos, f)
    sin = np.asarray(sin, f)
    wq, wk, wv, wo = (np.asarray(a, f) for a in (wq, wk, wv, wo))
    q_norm_w = np.asarray(q_norm_w, f)
    k_norm_w = np.asarray(k_norm_w, f)

    perm = head_perm()                      # partition p holds dim perm[p]
    partner = np.array([(p // 32) * 32 + ((p % 32) + 16) % 32
                        for p in range(D)])  # stream_shuffle pairing
    sign = np.where(perm[np.arange(D)] < D // 2, -1.0, 1.0).astype(f)
    # rot_half weight fold: t1[p] = qk[partner(p)] * stq[p];
    # stq[p] = sign(d_p) * sin[d_p] * w[d at partner]
    xt = np.ascontiguousarray(x.reshape(T, HID).T)  # [HID, T]
    ctq = cos.T[perm] * q_norm_w[perm][:, None]
    stq = sin.T[perm] * q_norm_w[perm[partner]][:, None] * sign[:, None]
    ctk = cos.T[perm] * k_norm_w[perm][:, None]
    stk = sin.T[perm] * k_norm_w[perm[partner]][:, None] * sign[:, None]
    onec = np.ones((D, 1), f)
    xt_m, ctq_m, stq_m, ctk_m, stk_m, onec_m = (
        cvt(a) for a in (xt, ctq, stq, ctk, stk, onec))

    in_maps = []
    for c in range(NCORES):
        wq_c = wq[:, c * HQ * D:(c + 1) * HQ * D].reshape(HID, HQ, D)
        wq_c = wq_c[:, :, perm].reshape(HID, HQ * D)  # permuted head dims
        wk_c = wk[:, c * D:(c + 1) * D][:, perm]
        wv_c = wv[:, c * D:(c + 1) * D]               # v unpermuted
        wqkv_c = np.ascontiguousarray(
            np.concatenate([wq_c, wk_c, wv_c], axis=1))
        woc = np.ascontiguousarray(wo[c * HQ * D:(c + 1) * HQ * D, :])
        in_maps.append({
            "xt": xt_m, "wqkv": cvt(wqkv_c), "woc": cvt(woc), "onec": onec_m,
            "ctq": ctq_m, "stq": stq_m, "ctk": ctk_m, "stk": stk_m,
        })
    return in_maps


_NC = None


def get_nc():
    global _NC
    if _NC is None:
        _NC = build_nc()
    return _NC


def kernel(x, cos, sin, wq, wk, wv, wo, q_norm_w, k_norm_w):
    nc = get_nc()
    in_maps = prep_inputs(x, cos, sin, wq, wk, wv, wo, q_norm_w, k_norm_w)
    res = run_bass_kernel_spmd(nc, in_maps, core_ids=list(range(NCORES)))
    acc = np.zeros((T, HID), dtype=np.float64)
    for c in range(NCORES):
        acc += res.results[c]["out"]
    return acc.astype(np.float32).reshape(B, S, HID)


# revision 18
# speedup vs baseline: 1.0272x; 1.0084x over previous
"""Trainium2 Bass kernel for a GQA attention block (B=2, S=2048, H=2048,
16 q-heads / 8 kv-heads, head_dim=128, fp32), tensor-parallel over heads
across 8 NeuronCores.

Per-core shard (core c): q-heads {2c, 2c+1}, kv-head c; wq/wk/wv column
shards, wo row shard. x replicated (pre-transposed on host). Each core
emits a partial [4096, 2048] o-proj product; host sums the 8 partials.

v2 design (all matmul I/O in bf16; graded by the TimelineSim cost model):
  - Emission order A(b0) B(b0) A(b1) B(b1): batch-b attention starts as
    soon as batch-b projections finish; b1's input DMA prefetches under
    B(b0) (loads on the sync queue, output stores on the scalar queue).
  - Phase A per 512-token tile: Q^T/K^T projection slabs [d,tok] plus V
    produced directly in natural [tok,d] layout (x^T chunks stationary),
    so no PE transposes. Rope rotate-half runs on DVE stream_shuffle:
    the head dim is host-permuted so (d, d+64) pairs sit in the same
    32-partition block (within-block shuffle is all HW supports); the
    rotation sign is folded into the sin tables. RMSNorm rstd applied
    post-rope (column-uniform, commutes); the whole mul chain is bf16
    for DVE 2x throughput.
  - Phase B per (b, q-tile): scores S^T [k,q] per 128-k tile; exp on ACT
    (4 k-tiles per instruction); causal masking via gpsimd affine_select
    on the two diagonal-band tiles; softmax denominator as ap=1 matmuls
    (exp tile stationary, ones moving -> [q,1] PSUM columns, ~free on
    the PE); PV accumulated [d,q]; o-proj per head into separate PSUM
    halves, normalization fused into eviction as per-partition scales
    (ACT/DVE scale + DVE/Pool scalar_tensor_tensor accumulate).
"""

import math
import os
import sys

import numpy as np

for _p in ("/opt/trn_rl_repo", "/root/.axon_site/_ro/trn_rl_repo"):
    if os.path.isdir(_p) and _p not in sys.path:
        sys.path.insert(0, _p)
        break

import concourse.bacc as bacc
import concourse.tile as tile
from concourse import mybir
from concourse.bass_isa import ReduceOp
from concourse.bass_utils import run_bass_kernel_spmd
from concourse.masks import make_identity

# Problem constants (hardcoded per contract)
B, S, HID = 2, 2048, 2048
NH, NKV, D = 16, 8, 128
NCORES = 8
HQ = NH // NCORES  # q heads per core = 2
T = B * S          # 4096 tokens
EPS = 1e-5
F32 = mybir.dt.float32
BF16 = mybir.dt.bfloat16
SCALE = 1.0 / math.sqrt(D)

KT = HID // 128      # 16 contraction tiles
QT_PER_B = S // 512  # 4 q-tiles per batch

# within-32-block half-rotation (self-inverse); with the host-side head-dim
# permutation below this realizes rotate-half across the full 128 dims
SHUF_MASK = [(i + 16) % 32 for i in range(32)]


def head_perm():
    """p -> d: partition p holds original head-dim d; pairs (d, d+64) share
    a 32-partition block 16 apart, so stream_shuffle can swap them."""
    perm = np.zeros(D, dtype=np.int64)
    for p in range(D):
        blk, i = p // 32, p % 32
        perm[p] = 16 * blk + (i % 16) + 64 * (i // 16)
    return perm


def build_nc():
    nc = bacc.Bacc("TRN2", target_bir_lowering=False, debug=False)
    xt = nc.dram_tensor("xt", [HID, T], BF16, kind="ExternalInput").ap()
    wqkv = nc.dram_tensor("wqkv", [HID, 4 * D], BF16, kind="ExternalInput").ap()
    woc = nc.dram_tensor("woc", [HQ * D, HID], BF16, kind="ExternalInput").ap()
    onec = nc.dram_tensor("onec", [D, 1], BF16, kind="ExternalInput").ap()
    ctq = nc.dram_tensor("ctq", [D, S], BF16, kind="ExternalInput").ap()
    stq = nc.dram_tensor("stq", [D, S], BF16, kind="ExternalInput").ap()
    ctk = nc.dram_tensor("ctk", [D, S], BF16, kind="ExternalInput").ap()
    stk = nc.dram_tensor("stk", [D, S], BF16, kind="ExternalInput").ap()
    out = nc.dram_tensor("out", [T, HID], BF16, kind="ExternalOutput").ap()

    with tile.TileContext(nc) as tc:
        from contextlib import ExitStack

        with ExitStack() as root:
            const = root.enter_context(tc.tile_pool(name="const", bufs=1))
            ident = const.tile([128, 128], F32, name="ident")
            make_identity(nc, ident)
            ones_col = const.tile([128, 1], BF16, name="ones_col")
            nc.scalar.dma_start(out=ones_col, in_=onec)
            eps_col = const.tile([128, 1], F32, name="eps_col")
            nc.vector.memset(eps_col, EPS)

            res = root.enter_context(tc.tile_pool(name="res", bufs=1))
            wo_sb = res.tile([128, HQ, HID], BF16, name="wo_sb")
            wqkv_sb = res.tile([128, KT, 4 * D], BF16, name="wqkv_sb")
            qt_sb = res.tile([128, HQ, T], BF16, name="qt_sb")   # [d, h, tok]
            kt_sb = res.tile([128, T], BF16, name="kt_sb")       # [d, tok]
            v_sb = res.tile([128, T // 128, D], BF16, name="v_sb")
            tabs = {}
            for nm in ("cq", "sq", "ck", "sk"):
                tabs[nm] = res.tile([128, S], BF16, name="tab_" + nm)

            xp = root.enter_context(tc.tile_pool(name="xp", bufs=18))
            wp = root.enter_context(tc.tile_pool(name="wp", bufs=2))
            ep = root.enter_context(tc.tile_pool(name="ep", bufs=19))
            atp = root.enter_context(tc.tile_pool(name="atp", bufs=8))
            rdp = root.enter_context(tc.tile_pool(name="rdp", bufs=8))
            rrp = root.enter_context(tc.tile_pool(name="rrp", bufs=4))
            rbp = root.enter_context(tc.tile_pool(name="rbp", bufs=4))
            op = root.enter_context(tc.tile_pool(name="op", bufs=4))
            psB = root.enter_context(tc.tile_pool(name="psB", bufs=2, space="PSUM"))
            psPo = root.enter_context(tc.tile_pool(name="psPo", bufs=2, space="PSUM"))
            psO = root.enter_context(tc.tile_pool(name="psO", bufs=2, space="PSUM"))

            def phase_a(b):
                """Projections + norm + rope for batch b's 4 token tiles."""
                xhs = {}

                def load_x(k, half):
                    xh = xp.tile([128, 1024], BF16, name="xh", tag="xh")
                    nc.sync.dma_start(
                        out=xh,
                        in_=xt[k * 128:(k + 1) * 128,
                               b * S + half * 1024: b * S + (half + 1) * 1024],
                    )
                    xhs[(k, half)] = xh

                # first-half x loads, with weight loads interleaved (b==0) so
                # the first slab's deps resolve early
                for k0 in range(0, KT, 4):
                    if b == 0:
                        nc.sync.dma_start(
                            out=wqkv_sb[:, k0:k0 + 4, :],
                            in_=wqkv[k0 * 128:(k0 + 4) * 128, :].rearrange(
                                "(k p) n -> p k n", p=128),
                        )
                    for k in range(k0, k0 + 4):
                        load_x(k, 0)
                if b == 0:
                    for nm, ap in (("cq", ctq), ("sq", stq), ("ck", ctk),
                                   ("sk", stk)):
                        nc.sync.dma_start(out=tabs[nm], in_=ap)
                    nc.sync.dma_start(
                        out=wo_sb, in_=woc.rearrange("(h p) n -> p h n", p=128))
                for k in range(KT):
                    load_x(k, 1)
                for tl in range(4):  # local 512-token tile
                    t = b * 4 + tl
                    xks = [xhs[(k, tl // 2)][:, (tl % 2) * 512:(tl % 2 + 1) * 512]
                           for k in range(KT)]
                    # two PSUM slabs: (q0,q1) and (k, v-natural)
                    big_q = psB.tile([128, 1024], F32, name="ps_q", tag="ps")
                    for m in range(2):
                        dst = big_q[:, m * 512:(m + 1) * 512]
                        for k in range(KT):
                            nc.tensor.matmul(
                                dst, lhsT=wqkv_sb[:, k, m * 128:(m + 1) * 128],
                                rhs=xks[k], start=(k == 0), stop=(k == KT - 1),
                            )
                    big_kv = psB.tile([128, 1024], F32, name="ps_kv", tag="ps")
                    for k in range(KT):
                        nc.tensor.matmul(
                            big_kv[:, 0:512], lhsT=wqkv_sb[:, k, 256:384],
                            rhs=xks[k], start=(k == 0), stop=(k == KT - 1),
                        )
                    for k in range(KT):
                        for j in range(4):
                            nc.tensor.matmul(
                                big_kv[:, 512 + j * 128:512 + (j + 1) * 128],
                                lhsT=xks[k][:, j * 128:(j + 1) * 128],
                                rhs=wqkv_sb[:, k, 384:512],
                                start=(k == 0), stop=(k == KT - 1),
                                skip_group_check=True,
                            )
                    nc.scalar.copy(v_sb[:, t * 4:(t + 1) * 4, :],
                                   big_kv[:, 512:1024])

                    s0 = tl * 512  # position-in-sequence
                    for m, src, cosT, sinT in (
                        (0, big_q[:, 0:512], tabs["cq"], tabs["sq"]),
                        (1, big_q[:, 512:1024], tabs["cq"], tabs["sq"]),
                        (2, big_kv[:, 0:512], tabs["ck"], tabs["sk"]),
                    ):
                        cosT = cosT[:, s0:s0 + 512]
                        sinT = sinT[:, s0:s0 + 512]
                        qk = wp.tile([128, 512], BF16, name="qk", tag="qk")
                        nc.scalar.copy(qk, src)  # sole PSUM reader (ACT)
                        sq = wp.tile([128, 512], BF16, name="sqr", tag="sqr")
                        nc.vector.tensor_mul(sq, qk, qk)
                        nc.gpsimd.partition_all_reduce(sq, sq, 128, ReduceOp.add)
                        rrow = wp.tile([1, 512], F32, name="rrow", tag="rrow")
                        nc.scalar.activation(
                            rrow, sq[0:1, :], mybir.ActivationFunctionType.Sqrt,
                            bias=eps_col[0:1, :], scale=1.0 / D,
                        )
                        rrow_r = wp.tile([1, 512], BF16, name="rrow_r", tag="rrow_r")
                        with nc.allow_low_precision(
                                reason="rstd in bf16; rel-err budget 2e-2"):
                            nc.vector.reciprocal(rrow_r, rrow)
                        rstd = wp.tile([128, 512], BF16, name="rstd", tag="rstd")
                        nc.gpsimd.partition_broadcast(rstd, rrow_r)
                        shf = wp.tile([128, 512], BF16, name="shf", tag="shf")
                        nc.vector.stream_shuffle(shf, qk, SHUF_MASK)
                        t0 = wp.tile([128, 512], BF16, name="t0", tag="t0")
                        nc.vector.tensor_mul(t0, qk, cosT)
                        t1 = wp.tile([128, 512], BF16, name="t1", tag="t1")
                        nc.vector.tensor_mul(t1, shf, sinT)
                        tr = wp.tile([128, 512], BF16, name="tr", tag="tr")
                        nc.vector.tensor_add(tr, t0, t1)
                        if m < 2:
                            dst = qt_sb[:, m, t * 512:(t + 1) * 512]
                        else:
                            dst = kt_sb[:, t * 512:(t + 1) * 512]
                        nc.vector.tensor_mul(dst, tr, rstd)

            def phase_b(b):
                """Causal attention + row-parallel o-proj partial, batch b.

                Software-pipelined: group i+1's scores are emitted before
                group i's PV so the in-order PE never idles on the ACT exp
                latency; each q-tile's o-proj is deferred behind the next
                group's scores for the same reason.
                """
                at_tiles = {}
                rd_tiles = {}

                def scores(qt, h, qh):
                    qq0 = qt * 512 + qh * 256
                    n_kt = (qq0 + 256) // 128
                    ets = []
                    for k0 in range(0, n_kt, 4):
                        kn = min(4, n_kt - k0)
                        st = psB.tile([128, 1024], F32, name="st", tag="ps")
                        for j in range(kn):
                            kt = k0 + j
                            nc.tensor.matmul(
                                st[:, j * 256:(j + 1) * 256],
                                lhsT=kt_sb[:, b * S + kt * 128:
                                           b * S + (kt + 1) * 128],
                                rhs=qt_sb[:, h, b * S + qq0: b * S + qq0 + 256],
                                start=True, stop=True,
                                skip_group_check=True,
                            )
                        et = ep.tile([128, 1024], BF16, name="et", tag="et")
                        nc.scalar.activation(
                            et[:, 0:kn * 256], st[:, 0:kn * 256],
                            mybir.ActivationFunctionType.Exp, scale=SCALE,
                        )
                        for j in range(kn):
                            kt = k0 + j
                            esl = et[:, j * 256:(j + 1) * 256]
                            if kt * 128 + 127 > qq0:  # diagonal band
                                nc.gpsimd.affine_select(
                                    out=esl, in_=esl,
                                    pattern=[[1, 256]],
                                    channel_multiplier=-1,
                                    base=-(kt * 128 - qq0),
                                    compare_op=mybir.AluOpType.is_ge,
                                    fill=0.0,
                                )
                            ets.append(esl)
                    return ets

                def pv(qt, h, qh, ets):
                    """PV + den matmuls + reciprocal; returns a finisher that
                    normalizes the PV output into at_tiles. The finisher is
                    deferred one group so the PE transpose and the DVE/Pool
                    chain never head-of-line block the PE."""
                    n_kt = len(ets)
                    od = psO.tile([128, 512], F32, name="od", tag="od")
                    for kt in range(n_kt):
                        nc.tensor.matmul(
                            od[:, 0:256],
                            lhsT=v_sb[:, b * (S // 128) + kt, :],
                            rhs=ets[kt],
                            start=(kt == 0), stop=(kt == n_kt - 1),
                            skip_group_check=True,
                        )
                        for c in range(2):
                            nc.tensor.matmul(
                                od[:, 256 + c:257 + c],
                                lhsT=ets[kt][:, c * 128:(c + 1) * 128],
                                rhs=ones_col,
                                start=(kt == 0), stop=(kt == n_kt - 1),
                                skip_group_check=True,
                            )
                    r_t = rdp.tile([128, 2], F32, name="rd", tag="rd")
                    nc.vector.reciprocal(r_t, od[:, 256:258])

                    def fin():
                        # den cols are consumed; overwrite with rows rr[1,256]
                        for c in range(2):
                            nc.tensor.transpose(
                                od[0:1, 256 + c * 128:384 + c * 128],
                                r_t[:, c:c + 1], ident)
                        rr = rrp.tile([1, 256], F32, name="rr", tag="rr")
                        nc.vector.tensor_copy(rr, od[0:1, 256:512])
                        rb = rbp.tile([128, 256], F32, name="rb", tag="rb")
                        for c in range(2):
                            nc.gpsimd.partition_broadcast(
                                rb[:, c * 128:(c + 1) * 128],
                                rr[:, c * 128:(c + 1) * 128])
                        a_t = atp.tile([128, 256], BF16, name="at", tag="at")
                        nc.gpsimd.tensor_mul(a_t, od[:, 0:256], rb)
                        at_tiles[(h, qh)] = a_t

                    return fin

                def oproj(qt, ats):
                    for mq in range(4):
                        qh, c = mq // 2, mq % 2
                        ob = op.tile([128, 2048], BF16, name="ob", tag="ob")
                        for nn in range(4):
                            po = psPo.tile([128, 512], F32, name="po", tag="po")
                            for h in range(HQ):
                                nc.tensor.matmul(
                                    po,
                                    lhsT=ats[(h, qh)][:, c * 128:(c + 1) * 128],
                                    rhs=wo_sb[:, h, nn * 512:(nn + 1) * 512],
                                    start=(h == 0), stop=(h == HQ - 1),
                                )
                            osl = ob[:, nn * 512:(nn + 1) * 512]
                            # alternate Pool/DVE so consecutive po evictions
                            # overlap and ACT keeps feeding exps to the PE
                            if nn % 2 == 0:
                                nc.gpsimd.tensor_copy(osl, po)
                            else:
                                nc.vector.tensor_copy(osl, po)
                        nc.scalar.dma_start(
                            out=out[b * S + qt * 512 + mq * 128:
                                    b * S + qt * 512 + (mq + 1) * 128, :],
                            in_=ob,
                        )

                # super-iteration pipeline: oproj(qt-2) | per-group scores(qt)
                # + finisher(prev group) + pv(qt-1). Every engine's in-order
                # stream stays in true execution order: late-dep work (oproj
                # evictions, finisher chains) always sits behind the PE work
                # it depends on, so no head-of-line blocking.
                groups = [(h, qh) for h in range(HQ) for qh in range(2)]
                pend_ets = {}
                pend_fin = None
                for qt in range(QT_PER_B + 2):
                    if qt >= 2 and pend_fin is not None:
                        pend_fin()  # last fin of qt-1's pvs — completes
                        pend_fin = None  # the qt-2 at set early
                    for gi, g in enumerate(groups):
                        if qt < QT_PER_B:
                            pend_ets[(qt, *g)] = scores(qt, *g)
                        new_fin = None
                        if 1 <= qt <= QT_PER_B:
                            new_fin = pv(qt - 1, *g, pend_ets.pop((qt - 1, *g)))
                        # fin(g-1) emits after pv(g)'s matmuls: its PE
                        # transposes then never wait on the DVE reciprocal
                        if pend_fin is not None:
                            pend_fin()
                        pend_fin = new_fin
                        if gi == 0 and qt >= 2:
                            # a full group after the at set completed, so the
                            # po matmuls never wait on the last at-mul
                            oproj(qt - 2, at_tiles)

            for b in range(B):
                phase_a(b)
                phase_b(b)

    nc.compile()
    return nc


def prep_inputs(x, cos, sin, wq, wk, wv, wo, q_norm_w, k_norm_w):
    """Host-side sharding/layout prep. Returns per-core in_maps."""
    import ml_dtypes
    f = np.float32
    bf = np.dtype(ml_dtypes.bfloat16)
    cvt = lambda a: np.ascontiguousarray(np.asarray(a, f).astype(bf))
    x = np.asarray(x, f)
    cos = np.asarray(cos, f)
    sin = np.asarray(sin, f)
    wq, wk, wv, wo = (np.asarray(a, f) for a in (wq, wk, wv, wo))
    q_norm_w = np.asarray(q_norm_w, f)
    k_norm_w = np.asarray(k_norm_w, f)

    perm = head_perm()                      # partition p holds dim perm[p]
    partner = np.array([(p // 32) * 32 + ((p % 32) + 16) % 32
                        for p in range(D)])  # stream_shuffle pairing
    sign = np.where(perm[np.arange(D)] < D // 2, -1.0, 1.0).astype(f)
    # rot_half weight fold: t1[p] = qk[partner(p)] * stq[p];
    # stq[p] = sign(d_p) * sin[d_p] * w[d at partner]
    xt = np.ascontiguousarray(x.reshape(T, HID).T)  # [HID, T]
    ctq = cos.T[perm] * q_norm_w[perm][:, None]
    stq = sin.T[perm] * q_norm_w[perm[partner]][:, None] * sign[:, None]
    ctk = cos.T[perm] * k_norm_w[perm][:, None]
    stk = sin.T[perm] * k_norm_w[perm[partner]][:, None] * sign[:, None]
    onec = np.ones((D, 1), f)
    xt_m, ctq_m, stq_m, ctk_m, stk_m, onec_m = (
        cvt(a) for a in (xt, ctq, stq, ctk, stk, onec))

    in_maps = []
    for c in range(NCORES):
        wq_c = wq[:, c * HQ * D:(c + 1) * HQ * D].reshape(HID, HQ, D)
        wq_c = wq_c[:, :, perm].reshape(HID, HQ * D)  # permuted head dims
        wk_c = wk[:, c * D:(c + 1) * D][:, perm]
        wv_c = wv[:, c * D:(c + 1) * D]               # v unpermuted
        wqkv_c = np.ascontiguousarray(
            np.concatenate([wq_c, wk_c, wv_c], axis=1))
        woc = np.ascontiguousarray(wo[c * HQ * D:(c + 1) * HQ * D, :])
        in_maps.append({
            "xt": xt_m, "wqkv": cvt(wqkv_c), "woc": cvt(woc), "onec": onec_m,
            "ctq": ctq_m, "stq": stq_m, "ctk": ctk_m, "stk": stk_m,
        })
    return in_maps


_NC = None


def get_nc():
    global _NC
    if _NC is None:
        _NC = build_nc()
    return _NC


def kernel(x, cos, sin, wq, wk, wv, wo, q_norm_w, k_norm_w):
    nc = get_nc()
    in_maps = prep_inputs(x, cos, sin, wq, wk, wv, wo, q_norm_w, k_norm_w)
    res = run_bass_kernel_spmd(nc, in_maps, core_ids=list(range(NCORES)))
    acc = np.zeros((T, HID), dtype=np.float64)
    for c in range(NCORES):
        acc += np.asarray(res.results[c]["out"], dtype=np.float64)
    return acc.astype(np.float32).reshape(B, S, HID)


# revision 20
# speedup vs baseline: 1.0656x; 1.0374x over previous
"""Trainium2 Bass kernel for a GQA attention block (B=2, S=2048, H=2048,
16 q-heads / 8 kv-heads, head_dim=128, fp32), tensor-parallel over heads
across 8 NeuronCores.

Per-core shard (core c): q-heads {2c, 2c+1}, kv-head c; wq/wk/wv column
shards, wo row shard. x replicated (pre-transposed on host). Each core
emits a partial [4096, 2048] o-proj product; host sums the 8 partials.

v2 design (all matmul I/O in bf16; graded by the TimelineSim cost model):
  - Emission order A(b0) B(b0) A(b1) B(b1): batch-b attention starts as
    soon as batch-b projections finish; b1's input DMA prefetches under
    B(b0) (loads on the sync queue, output stores on the scalar queue).
  - Phase A per 512-token tile: Q^T/K^T projection slabs [d,tok] plus V
    produced directly in natural [tok,d] layout (x^T chunks stationary),
    so no PE transposes. Rope rotate-half runs on DVE stream_shuffle:
    the head dim is host-permuted so (d, d+64) pairs sit in the same
    32-partition block (within-block shuffle is all HW supports); the
    rotation sign is folded into the sin tables. RMSNorm rstd applied
    post-rope (column-uniform, commutes); the whole mul chain is bf16
    for DVE 2x throughput.
  - Phase B per (b, q-tile): scores S^T [k,q] per 128-k tile; exp on ACT
    (4 k-tiles per instruction); causal masking via gpsimd affine_select
    on the two diagonal-band tiles; softmax denominator as ap=1 matmuls
    (exp tile stationary, ones moving -> [q,1] PSUM columns, ~free on
    the PE); PV accumulated [d,q]; o-proj per head into separate PSUM
    halves, normalization fused into eviction as per-partition scales
    (ACT/DVE scale + DVE/Pool scalar_tensor_tensor accumulate).
"""

import math
import os
import sys

import numpy as np

for _p in ("/opt/trn_rl_repo", "/root/.axon_site/_ro/trn_rl_repo"):
    if os.path.isdir(_p) and _p not in sys.path:
        sys.path.insert(0, _p)
        break

import concourse.bacc as bacc
import concourse.tile as tile
from concourse import mybir
from concourse.bass_isa import ReduceOp
from concourse.bass_utils import run_bass_kernel_spmd
from concourse.masks import make_identity

# Problem constants (hardcoded per contract)
B, S, HID = 2, 2048, 2048
NH, NKV, D = 16, 8, 128
NCORES = 8
HQ = NH // NCORES  # q heads per core = 2
T = B * S          # 4096 tokens
EPS = 1e-5
F32 = mybir.dt.float32
BF16 = mybir.dt.bfloat16
SCALE = 1.0 / math.sqrt(D)

KT = HID // 128      # 16 contraction tiles
QT_PER_B = S // 512  # 4 q-tiles per batch

# within-32-block half-rotation (self-inverse); with the host-side head-dim
# permutation below this realizes rotate-half across the full 128 dims
SHUF_MASK = [(i + 16) % 32 for i in range(32)]


def head_perm():
    """p -> d: partition p holds original head-dim d; pairs (d, d+64) share
    a 32-partition block 16 apart, so stream_shuffle can swap them."""
    perm = np.zeros(D, dtype=np.int64)
    for p in range(D):
        blk, i = p // 32, p % 32
        perm[p] = 16 * blk + (i % 16) + 64 * (i // 16)
    return perm


def build_nc():
    nc = bacc.Bacc("TRN2", target_bir_lowering=False, debug=False)
    xt = nc.dram_tensor("xt", [HID, T], BF16, kind="ExternalInput").ap()
    wqkv = nc.dram_tensor("wqkv", [HID, 4 * D], BF16, kind="ExternalInput").ap()
    woc = nc.dram_tensor("woc", [HQ * D, HID], BF16, kind="ExternalInput").ap()
    onec = nc.dram_tensor("onec", [D, 1], BF16, kind="ExternalInput").ap()
    ctq = nc.dram_tensor("ctq", [D, S], BF16, kind="ExternalInput").ap()
    stq = nc.dram_tensor("stq", [D, S], BF16, kind="ExternalInput").ap()
    ctk = nc.dram_tensor("ctk", [D, S], BF16, kind="ExternalInput").ap()
    stk = nc.dram_tensor("stk", [D, S], BF16, kind="ExternalInput").ap()
    out = nc.dram_tensor("out", [T, HID], BF16, kind="ExternalOutput").ap()

    with tile.TileContext(nc) as tc:
        from contextlib import ExitStack

        with ExitStack() as root:
            const = root.enter_context(tc.tile_pool(name="const", bufs=1))
            ident = const.tile([128, 128], F32, name="ident")
            make_identity(nc, ident)
            ones_col = const.tile([128, 1], BF16, name="ones_col")
            nc.scalar.dma_start(out=ones_col, in_=onec)
            eps_col = const.tile([128, 1], F32, name="eps_col")
            nc.vector.memset(eps_col, EPS)

            res = root.enter_context(tc.tile_pool(name="res", bufs=1))
            wo_sb = res.tile([128, HQ, HID], BF16, name="wo_sb")
            wqkv_sb = res.tile([128, KT, 4 * D], BF16, name="wqkv_sb")
            qt_sb = res.tile([128, HQ, T], BF16, name="qt_sb")   # [d, h, tok]
            kt_sb = res.tile([128, T], BF16, name="kt_sb")       # [d, tok]
            v_sb = res.tile([128, T // 128, D], BF16, name="v_sb")
            tabs = {}
            for nm in ("cq", "sq", "ck", "sk"):
                tabs[nm] = res.tile([128, S], BF16, name="tab_" + nm)

            xp = root.enter_context(tc.tile_pool(name="xp", bufs=18))
            wp = root.enter_context(tc.tile_pool(name="wp", bufs=2))
            ep = root.enter_context(tc.tile_pool(name="ep", bufs=19))
            atp = root.enter_context(tc.tile_pool(name="atp", bufs=8))
            rdp = root.enter_context(tc.tile_pool(name="rdp", bufs=8))
            rrp = root.enter_context(tc.tile_pool(name="rrp", bufs=4))
            rbp = root.enter_context(tc.tile_pool(name="rbp", bufs=4))
            op = root.enter_context(tc.tile_pool(name="op", bufs=4))
            psB = root.enter_context(tc.tile_pool(name="psB", bufs=4, space="PSUM"))
            psPo = root.enter_context(tc.tile_pool(name="psPo", bufs=2, space="PSUM"))
            psO = root.enter_context(tc.tile_pool(name="psO", bufs=2, space="PSUM"))

            def phase_a(b):
                """Projections + norm + rope for batch b's 4 token tiles."""
                xhs = {}

                def load_x(k, half):
                    xh = xp.tile([128, 1024], BF16, name="xh", tag="xh")
                    nc.sync.dma_start(
                        out=xh,
                        in_=xt[k * 128:(k + 1) * 128,
                               b * S + half * 1024: b * S + (half + 1) * 1024],
                    )
                    xhs[(k, half)] = xh

                # first-half x loads, with weight loads interleaved (b==0) so
                # the first slab's deps resolve early
                for k0 in range(0, KT, 4):
                    if b == 0:
                        nc.sync.dma_start(
                            out=wqkv_sb[:, k0:k0 + 4, :],
                            in_=wqkv[k0 * 128:(k0 + 4) * 128, :].rearrange(
                                "(k p) n -> p k n", p=128),
                        )
                    for k in range(k0, k0 + 4):
                        load_x(k, 0)
                if b == 0:
                    for nm, ap in (("cq", ctq), ("sq", stq), ("ck", ctk),
                                   ("sk", stk)):
                        nc.sync.dma_start(out=tabs[nm], in_=ap)
                    nc.sync.dma_start(
                        out=wo_sb, in_=woc.rearrange("(h p) n -> p h n", p=128))
                for k in range(KT):
                    load_x(k, 1)
                for tl in range(4):  # local 512-token tile
                    t = b * 4 + tl
                    xks = [xhs[(k, tl // 2)][:, (tl % 2) * 512:(tl % 2 + 1) * 512]
                           for k in range(KT)]
                    # four 1-bank PSUM slabs: q0, q1, k, v-natural
                    slabs = {}
                    for m in range(3):
                        ps = psB.tile([128, 512], F32, name="ps_a", tag="ps")
                        for k in range(KT):
                            nc.tensor.matmul(
                                ps, lhsT=wqkv_sb[:, k, m * 128:(m + 1) * 128],
                                rhs=xks[k], start=(k == 0), stop=(k == KT - 1),
                            )
                        slabs[m] = ps
                    ps_v = psB.tile([128, 512], F32, name="ps_a", tag="ps")
                    for k in range(KT):
                        for j in range(4):
                            nc.tensor.matmul(
                                ps_v[:, j * 128:(j + 1) * 128],
                                lhsT=xks[k][:, j * 128:(j + 1) * 128],
                                rhs=wqkv_sb[:, k, 384:512],
                                start=(k == 0), stop=(k == KT - 1),
                                skip_group_check=True,
                            )
                    nc.scalar.copy(v_sb[:, t * 4:(t + 1) * 4, :], ps_v)

                    s0 = tl * 512  # position-in-sequence
                    for m, src, cosT, sinT in (
                        (0, slabs[0], tabs["cq"], tabs["sq"]),
                        (1, slabs[1], tabs["cq"], tabs["sq"]),
                        (2, slabs[2], tabs["ck"], tabs["sk"]),
                    ):
                        cosT = cosT[:, s0:s0 + 512]
                        sinT = sinT[:, s0:s0 + 512]
                        qk = wp.tile([128, 512], BF16, name="qk", tag="qk")
                        nc.scalar.copy(qk, src)  # sole PSUM reader (ACT)
                        sq = wp.tile([128, 512], BF16, name="sqr", tag="sqr")
                        nc.vector.tensor_mul(sq, qk, qk)
                        nc.gpsimd.partition_all_reduce(sq, sq, 128, ReduceOp.add)
                        rrow = wp.tile([1, 512], F32, name="rrow", tag="rrow")
                        nc.scalar.activation(
                            rrow, sq[0:1, :], mybir.ActivationFunctionType.Sqrt,
                            bias=eps_col[0:1, :], scale=1.0 / D,
                        )
                        rrow_r = wp.tile([1, 512], BF16, name="rrow_r", tag="rrow_r")
                        with nc.allow_low_precision(
                                reason="rstd in bf16; rel-err budget 2e-2"):
                            nc.vector.reciprocal(rrow_r, rrow)
                        rstd = wp.tile([128, 512], BF16, name="rstd", tag="rstd")
                        nc.gpsimd.partition_broadcast(rstd, rrow_r)
                        shf = wp.tile([128, 512], BF16, name="shf", tag="shf")
                        nc.vector.stream_shuffle(shf, qk, SHUF_MASK)
                        t0 = wp.tile([128, 512], BF16, name="t0", tag="t0")
                        nc.vector.tensor_mul(t0, qk, cosT)
                        t1 = wp.tile([128, 512], BF16, name="t1", tag="t1")
                        nc.vector.tensor_mul(t1, shf, sinT)
                        tr = wp.tile([128, 512], BF16, name="tr", tag="tr")
                        nc.vector.tensor_add(tr, t0, t1)
                        if m < 2:
                            dst = qt_sb[:, m, t * 512:(t + 1) * 512]
                        else:
                            dst = kt_sb[:, t * 512:(t + 1) * 512]
                        nc.vector.tensor_mul(dst, tr, rstd)

            def phase_b(b):
                """Causal attention + row-parallel o-proj partial, batch b.

                Software-pipelined: group i+1's scores are emitted before
                group i's PV so the in-order PE never idles on the ACT exp
                latency; each q-tile's o-proj is deferred behind the next
                group's scores for the same reason.
                """
                at_tiles = {}
                rd_tiles = {}

                def scores(qt, h, qh):
                    qq0 = qt * 512 + qh * 256
                    n_kt = (qq0 + 256) // 128
                    ets = []
                    for k0 in range(0, n_kt, 2):
                        kn = min(2, n_kt - k0)
                        st = psB.tile([128, 512], F32, name="st", tag="ps")
                        for j in range(kn):
                            kt = k0 + j
                            nc.tensor.matmul(
                                st[:, j * 256:(j + 1) * 256],
                                lhsT=kt_sb[:, b * S + kt * 128:
                                           b * S + (kt + 1) * 128],
                                rhs=qt_sb[:, h, b * S + qq0: b * S + qq0 + 256],
                                start=True, stop=True,
                                skip_group_check=True,
                            )
                        et = ep.tile([128, 512], BF16, name="et", tag="et")
                        nc.scalar.activation(
                            et[:, 0:kn * 256], st[:, 0:kn * 256],
                            mybir.ActivationFunctionType.Exp, scale=SCALE,
                        )
                        for j in range(kn):
                            kt = k0 + j
                            esl = et[:, j * 256:(j + 1) * 256]
                            if kt * 128 + 127 > qq0:  # diagonal band
                                nc.gpsimd.affine_select(
                                    out=esl, in_=esl,
                                    pattern=[[1, 256]],
                                    channel_multiplier=-1,
                                    base=-(kt * 128 - qq0),
                                    compare_op=mybir.AluOpType.is_ge,
                                    fill=0.0,
                                )
                            ets.append(esl)
                    return ets

                def pv(qt, h, qh, ets):
                    """PV + den matmuls + reciprocal; returns a finisher that
                    normalizes the PV output into at_tiles. The finisher is
                    deferred one group so the PE transpose and the DVE/Pool
                    chain never head-of-line block the PE."""
                    n_kt = len(ets)
                    od = psO.tile([128, 512], F32, name="od", tag="od")
                    for kt in range(n_kt):
                        nc.tensor.matmul(
                            od[:, 0:256],
                            lhsT=v_sb[:, b * (S // 128) + kt, :],
                            rhs=ets[kt],
                            start=(kt == 0), stop=(kt == n_kt - 1),
                            skip_group_check=True,
                        )
                        for c in range(2):
                            nc.tensor.matmul(
                                od[:, 256 + c:257 + c],
                                lhsT=ets[kt][:, c * 128:(c + 1) * 128],
                                rhs=ones_col,
                                start=(kt == 0), stop=(kt == n_kt - 1),
                                skip_group_check=True,
                            )
                    r_t = rdp.tile([128, 2], F32, name="rd", tag="rd")
                    nc.vector.reciprocal(r_t, od[:, 256:258])

                    def fin():
                        # den cols are consumed; overwrite with rows rr[1,256]
                        for c in range(2):
                            nc.tensor.transpose(
                                od[0:1, 256 + c * 128:384 + c * 128],
                                r_t[:, c:c + 1], ident)
                        rr = rrp.tile([1, 256], F32, name="rr", tag="rr")
                        nc.vector.tensor_copy(rr, od[0:1, 256:512])
                        rb = rbp.tile([128, 256], F32, name="rb", tag="rb")
                        for c in range(2):
                            nc.gpsimd.partition_broadcast(
                                rb[:, c * 128:(c + 1) * 128],
                                rr[:, c * 128:(c + 1) * 128])
                        a_t = atp.tile([128, 256], BF16, name="at", tag="at")
                        nc.gpsimd.tensor_mul(a_t, od[:, 0:256], rb)
                        at_tiles[(h, qh)] = a_t

                    return fin

                def oproj(qt, ats):
                    for mq in range(4):
                        qh, c = mq // 2, mq % 2
                        ob = op.tile([128, 2048], BF16, name="ob", tag="ob")
                        for nn in range(4):
                            po = psPo.tile([128, 512], F32, name="po", tag="po")
                            for h in range(HQ):
                                nc.tensor.matmul(
                                    po,
                                    lhsT=ats[(h, qh)][:, c * 128:(c + 1) * 128],
                                    rhs=wo_sb[:, h, nn * 512:(nn + 1) * 512],
                                    start=(h == 0), stop=(h == HQ - 1),
                                )
                            osl = ob[:, nn * 512:(nn + 1) * 512]
                            # alternate Pool/DVE so consecutive po evictions
                            # overlap and ACT keeps feeding exps to the PE
                            if nn % 2 == 0:
                                nc.gpsimd.tensor_copy(osl, po)
                            else:
                                nc.vector.tensor_copy(osl, po)
                        nc.scalar.dma_start(
                            out=out[b * S + qt * 512 + mq * 128:
                                    b * S + qt * 512 + (mq + 1) * 128, :],
                            in_=ob,
                        )

                # super-iteration pipeline: oproj(qt-2) | per-group scores(qt)
                # + finisher(prev group) + pv(qt-1). Every engine's in-order
                # stream stays in true execution order: late-dep work (oproj
                # evictions, finisher chains) always sits behind the PE work
                # it depends on, so no head-of-line blocking.
                groups = [(h, qh) for h in range(HQ) for qh in range(2)]
                pend_ets = {}
                pend_fin = None
                for qt in range(QT_PER_B + 2):
                    if qt >= 2 and pend_fin is not None:
                        pend_fin()  # last fin of qt-1's pvs — completes
                        pend_fin = None  # the qt-2 at set early
                    for gi, g in enumerate(groups):
                        if qt < QT_PER_B:
                            pend_ets[(qt, *g)] = scores(qt, *g)
                        new_fin = None
                        if 1 <= qt <= QT_PER_B:
                            new_fin = pv(qt - 1, *g, pend_ets.pop((qt - 1, *g)))
                        # fin(g-1) emits after pv(g)'s matmuls: its PE
                        # transposes then never wait on the DVE reciprocal
                        if pend_fin is not None:
                            pend_fin()
                        pend_fin = new_fin
                        if gi == 1 and qt >= 2:
                            # a full group after the at set completed, so the
                            # po matmuls never wait on the last at-mul
                            oproj(qt - 2, at_tiles)

            for b in range(B):
                phase_a(b)
                phase_b(b)

    nc.compile()
    return nc


def prep_inputs(x, cos, sin, wq, wk, wv, wo, q_norm_w, k_norm_w):
    """Host-side sharding/layout prep. Returns per-core in_maps."""
    import ml_dtypes
    f = np.float32
    bf = np.dtype(ml_dtypes.bfloat16)
    cvt = lambda a: np.ascontiguousarray(np.asarray(a, f).astype(bf))
    x = np.asarray(x, f)
    cos = np.asarray(cos, f)
    sin = np.asarray(sin, f)
    wq, wk, wv, wo = (np.asarray(a, f) for a in (wq, wk, wv, wo))
    q_norm_w = np.asarray(q_norm_w, f)
    k_norm_w = np.asarray(k_norm_w, f)

    perm = head_perm()                      # partition p holds dim perm[p]
    partner = np.array([(p // 32) * 32 + ((p % 32) + 16) % 32
                        for p in range(D)])  # stream_shuffle pairing
    sign = np.where(perm[np.arange(D)] < D // 2, -1.0, 1.0).astype(f)
    # rot_half weight fold: t1[p] = qk[partner(p)] * stq[p];
    # stq[p] = sign(d_p) * sin[d_p] * w[d at partner]
    xt = np.ascontiguousarray(x.reshape(T, HID).T)  # [HID, T]
    ctq = cos.T[perm] * q_norm_w[perm][:, None]
    stq = sin.T[perm] * q_norm_w[perm[partner]][:, None] * sign[:, None]
    ctk = cos.T[perm] * k_norm_w[perm][:, None]
    stk = sin.T[perm] * k_norm_w[perm[partner]][:, None] * sign[:, None]
    onec = np.ones((D, 1), f)
    xt_m, ctq_m, stq_m, ctk_m, stk_m, onec_m = (
        cvt(a) for a in (xt, ctq, stq, ctk, stk, onec))

    in_maps = []
    for c in range(NCORES):
        wq_c = wq[:, c * HQ * D:(c + 1) * HQ * D].reshape(HID, HQ, D)
        wq_c = wq_c[:, :, perm].reshape(HID, HQ * D)  # permuted head dims
        wk_c = wk[:, c * D:(c + 1) * D][:, perm]
        wv_c = wv[:, c * D:(c + 1) * D]               # v unpermuted
        wqkv_c = np.ascontiguousarray(
            np.concatenate([wq_c, wk_c, wv_c], axis=1))
        woc = np.ascontiguousarray(wo[c * HQ * D:(c + 1) * HQ * D, :])
        in_maps.append({
            "xt": xt_m, "wqkv": cvt(wqkv_c), "woc": cvt(woc), "onec": onec_m,
            "ctq": ctq_m, "stq": stq_m, "ctk": ctk_m, "stk": stk_m,
        })
    return in_maps


_NC = None


def get_nc():
    global _NC
    if _NC is None:
        _NC = build_nc()
    return _NC


def kernel(x, cos, sin, wq, wk, wv, wo, q_norm_w, k_norm_w):
    nc = get_nc()
    in_maps = prep_inputs(x, cos, sin, wq, wk, wv, wo, q_norm_w, k_norm_w)
    res = run_bass_kernel_spmd(nc, in_maps, core_ids=list(range(NCORES)))
    acc = np.zeros((T, HID), dtype=np.float64)
    for c in range(NCORES):
        acc += np.asarray(res.results[c]["out"], dtype=np.float64)
    return acc.astype(np.float32).reshape(B, S, HID)
